# revision 12
# baseline (speedup 1.0000x reference)
"""Trainium2 Bass kernel for MineralFusion, v2 (quad-packed convs).

Key idea vs baseline: each fp8 DoubleRow matmul covers a 4-tap column-quad
(dy in {d,d+1,d+2,d+3}, fixed dx) for 64 channels of BOTH the fused 5x5 and
the 7x7 conv at once, via a channel-replica moving tile whose upper 64
partitions hold the same 64 channels pre-shifted by one image row (+PW).
DR step 128 (2 rows) covers dy = d, d+2 on the base copy and d+1, d+3 on the
shifted copy.  14 quad-mms per 64-ch sub-block per 512-col chunk replace the
~18 diag mms + 7 DVE taps of the baseline.  Output lanes: one sub-block's
c3 lands channel-aligned, the fused stream lands half-swapped (pi = XOR 64);
the pi order is carried through score conv / top-k / mask and undone by one
SBUF->SBUF DMA of the bf16 mask per tile.

Top-k: 4 rounds of max8 + 3 match_replace (into a scratch copy) give the
30th-largest value per row; mask = (scr >= t30) via one tensor_scalar pass.
"""
import numpy as np
import ml_dtypes

B, C, H, W = 32, 256, 56, 56
K = 30
N_CORES = 8
B_LOC = B // N_CORES
NBLK = C // 128

PW = 64
NROW = 62                     # 3 + 56 + 3 rows
PLANE = NROW * PW             # 3968
XF8_X = PLANE + PW            # xf8 carries slack for the +PW replica copy
XQ_X = PLANE + 8              # quad tiles: AP tail slack
XP_X = 3136
ORIG = 3 * PW + 4             # interior origin (row 3, col 4)
CMP = 3584                    # 7 chunks x 512 (448 data + 64 junk)
FLT = 3136                    # flat compact (no junk cols)
NEG_BIG = -(2.0 ** 100)

QCOLS = list(range(-3, 4))    # dx of quad columns
QBASE = (-3, 1)               # quad q covers dy in {d, d+1, d+2, d+3}

LAST = {}


def _pad_view(ap_flat, dy, dx):
    off = ORIG + dy * PW + dx
    v = ap_flat[:, off:off + 7 * 8 * PW]
    return v.rearrange("p (k r w) -> p k r w", k=7, r=8, w=PW)[:, :, :, :56]


def _cmp_half(ap_flat, nk):
    v = ap_flat.rearrange("p (k r w) -> p k r w", k=nk, r=8, w=64)
    return v[:, :, :, :56]


def _cmp_view(ap_flat):
    v = ap_flat.rearrange("p (k r w) -> p k r w", k=7, r=8, w=64)
    return v[:, :, :, :56]


def build_nc(b_loc=B_LOC, b3_nonzero=False, b12_nonzero=False):
    import concourse.bass as bass
    import concourse.mybir as mybir
    from concourse import bacc, tile

    f32 = mybir.dt.float32
    bf16 = mybir.dt.bfloat16
    fp8 = mybir.dt.float8e4
    AF = mybir.ActivationFunctionType
    OP = mybir.AluOpType

    ntiles = b_loc * NBLK

    nc = bacc.Bacc("TRN2", target_bir_lowering=False, debug=False)

    x_d = nc.declare_dram_parameter("x", [b_loc, C, H, W], f32, isOutput=False)
    # quad weights: [blk][sb][14 slots][2 dr][128 rows] -> as [128, NBLK*2*14*2*128]
    dgQ_d = nc.declare_dram_parameter("dgQ", [NBLK, 128, 2 * 14 * 2 * 128], fp8,
                                      isOutput=False)
    dgS_d = nc.declare_dram_parameter("dgS", [NBLK, 128, 9 * 128], fp8,
                                      isOutput=False)
    bf_d = nc.declare_dram_parameter("bfpi", [NBLK, 128, 1], f32, isOutput=False)
    b3_d = nc.declare_dram_parameter("b3p", [NBLK, 128, 1], f32, isOutput=False)
    s1_d = nc.declare_dram_parameter("sew1", [NBLK, 128, 16], f32, isOutput=False)
    s2_d = nc.declare_dram_parameter("sew2", [NBLK, 16, 128], f32, isOutput=False)
    out_d = nc.declare_dram_parameter("out", [b_loc, C, H, W], f32, isOutput=True)

    from contextlib import ExitStack
    with tile.TileContext(nc) as tc:
        with ExitStack() as stack:
            ep = stack.enter_context
            wpool = ep(tc.tile_pool(name="wpool", bufs=1))
            xp_pool = ep(tc.tile_pool(name="xp", bufs=2))
            xf8_pool = ep(tc.tile_pool(name="xf8", bufs=2))
            xq_pool = ep(tc.tile_pool(name="xq", bufs=1))
            c3t_pool = ep(tc.tile_pool(name="c3t", bufs=3))
            fus_pool = ep(tc.tile_pool(name="fus", bufs=3))
            fus8_pool = ep(tc.tile_pool(name="fus8", bufs=2))
            scr_pool = ep(tc.tile_pool(name="scr", bufs=3))
            scrw_pool = ep(tc.tile_pool(name="scrw", bufs=1))
            msk_pool = ep(tc.tile_pool(name="msk", bufs=1))
            mskT_pool = ep(tc.tile_pool(name="mskT", bufs=1))
            y0_pool = ep(tc.tile_pool(name="y0", bufs=6))
            sm_pool = ep(tc.tile_pool(name="small", bufs=8))
            gs_pool = ep(tc.tile_pool(name="gs", bufs=9))
            gate_pool = ep(tc.tile_pool(name="gate", bufs=6))
            hsb_pool = ep(tc.tile_pool(name="hsb", bufs=4))
            out_pool = ep(tc.tile_pool(name="outp", bufs=1))
            pepA_pool = ep(tc.tile_pool(name="pepA", bufs=1, space="PSUM"))
            pepB_pool = ep(tc.tile_pool(name="pepB", bufs=1, space="PSUM"))
            ps_pool = ep(tc.tile_pool(name="ps", bufs=1, space="PSUM"))
            dgQ_sb = wpool.tile([128, NBLK * 2 * 14 * 2 * 128], fp8)
            dgS_sb = wpool.tile([128, NBLK * 9 * 128], fp8)
            bf_sb = wpool.tile([128, NBLK], f32)
            b3_sb = wpool.tile([128, NBLK], f32)
            s1_sb = wpool.tile([128, NBLK * 16], f32)
            s2_sb = wpool.tile([16, NBLK * 128], f32)
            QW = 2 * 14 * 2 * 128
            for blk in range(NBLK):
                nc.sync.dma_start(out=dgQ_sb[:, blk * QW:(blk + 1) * QW],
                                  in_=dgQ_d[blk])
                nc.sync.dma_start(out=dgS_sb[:, blk * 9 * 128:(blk + 1) * 9 * 128],
                                  in_=dgS_d[blk])
                nc.sync.dma_start(out=bf_sb[:, blk:blk + 1], in_=bf_d[blk])
                nc.sync.dma_start(out=b3_sb[:, blk:blk + 1], in_=b3_d[blk])
                nc.sync.dma_start(out=s1_sb[:, blk * 16:(blk + 1) * 16], in_=s1_d[blk])
                nc.sync.dma_start(out=s2_sb[:, blk * 128:(blk + 1) * 128],
                                  in_=s2_d[blk])

            gsums = {}
            y0s = {}
            hsbs = {}
            done_a = set()
            done_b = set()

            def emit_se_a(t, bd):
                hp = ps_pool.tile([16, 1], f32, tag="sep", name=f"hp{t}")
                for b2 in range(NBLK):
                    nc.tensor.matmul(
                        hp[:], s1_sb[:, b2 * 16:(b2 + 1) * 16],
                        gsums[bd * NBLK + b2][:],
                        start=(b2 == 0), stop=(b2 == NBLK - 1))
                hsb = hsb_pool.tile([16, 1], f32, tag="hsb", name=f"hsb{t}")
                nc.scalar.activation(hsb[:], hp[:], AF.Relu)
                hsbs[bd] = hsb

            def emit_se_b(t, bd):
                hsb = hsbs[bd]
                for b2 in range(NBLK):
                    glp = ps_pool.tile([128, 1], f32, tag="sep", name=f"glp{t}_{b2}")
                    nc.tensor.matmul(
                        glp[:], s2_sb[:, b2 * 128:(b2 + 1) * 128], hsb[:],
                        start=True, stop=True)
                    gt = gate_pool.tile([128, 1], f32, tag="gate", name=f"gt{t}_{b2}")
                    nc.scalar.activation(gt[:], glp[:], AF.Sigmoid)
                    nc.vector.tensor_scalar_add(gt[:], gt[:], 1.0)
                    t2 = bd * NBLK + b2
                    dst = out_d[bd, b2 * 128:(b2 + 1) * 128] \
                        .rearrange("c h w -> c (h w)")
                    for hf in range(2):
                        outf = out_pool.tile([128, FLT // 2], f32, tag="outp",
                                             name=f"outf{t}_{b2}_{hf}")
                        sl = slice(hf * (FLT // 2), (hf + 1) * (FLT // 2))
                        nc.scalar.activation(outf[:], y0s[t2][:, sl],
                                             AF.Copy, bias=0.0, scale=gt[:])
                        nc.sync.dma_start(out=dst[:, sl], in_=outf[:])

            for t in range(ntiles):
                b, blk = divmod(t, NBLK)
                c0 = blk * 128

                xp = xp_pool.tile([128, XP_X], f32)
                x_src = x_d[b, c0:c0 + 128].rearrange("c h w -> c (h w)")
                nc.sync.dma_start(out=xp[:], in_=x_src)

                xf8 = xf8_pool.tile([128, XF8_X], fp8)
                nc.gpsimd.memset(xf8[:, PLANE:XF8_X], 0.0)
                nc.gpsimd.memset(xf8[:, 0:3 * PW], 0.0)
                nc.gpsimd.memset(xf8[:, 59 * PW:PLANE], 0.0)
                xcol = xf8[:, 3 * PW:59 * PW].rearrange("p (h w) -> p h w", w=PW)
                nc.gpsimd.memset(xcol[:, :, 0:4], 0.0)
                nc.gpsimd.memset(xcol[:, :, 60:64], 0.0)
                nc.scalar.activation(
                    _pad_view(xf8, 0, 0),
                    xp[:].rearrange("p (k r w) -> p k r w", k=7, r=8, w=56),
                    AF.Copy)

                # channel-replica quad tiles: xq[sb][0:64) = xf8 sub-block,
                # [64:128) = same shifted by one image row (+PW)
                xqs = []
                for sb in range(2):
                    xq = xq_pool.tile([128, XQ_X], fp8, tag=f"xq{sb}", name=f"xq{t}_{sb}")
                    nc.gpsimd.memset(xq[:, PLANE:XQ_X], 0.0)
                    nc.sync.dma_start(out=xq[0:64, 0:PLANE],
                                      in_=xf8[64 * sb:64 * sb + 64, 0:PLANE])
                    nc.sync.dma_start(out=xq[64:128, 0:PLANE],
                                      in_=xf8[64 * sb:64 * sb + 64, PW:PLANE + PW])
                    xqs.append(xq)

                # ---- quad conv phases: sb0 then sb1 ----
                fus = fus_pool.tile([128, FLT], bf16)
                c3t = c3t_pool.tile([128, FLT], bf16, tag="c3t", name=f"c3t{t}")
                y0 = y0_pool.tile([128, FLT], bf16)
                y0s[t] = y0
                pstep = xqs[0][:].ap[0][0]
                for sb in range(2):
                    xq = xqs[sb]
                    c3lo = 64 * sb            # c3 lanes: channel-aligned
                    fulo = 64 - 64 * sb       # fused lanes: swapped half
                    halves = []
                    for hi, (hpool, clo, chi, nk) in enumerate(
                            ((pepA_pool, 0, 4, 4), (pepB_pool, 4, 7, 3))):
                        pq = hpool.tile([128, nk * 512], f32, tag=f"pep{hi}",
                                        name=f"pq{t}_{sb}_{hi}")
                        halves.append((pq, clo, chi, nk))
                        nmm = 0
                        for x in QCOLS:
                            for qi, d in enumerate(QBASE):
                                base = (blk * 2 + sb) * 14 + x + 3 + 7 * qi
                                lhs = dgQ_sb[:, base * 256:(base + 1) * 256] \
                                    .rearrange("p (i m) -> p i m", i=2, m=128)
                                off0 = ORIG + d * PW + x
                                for ch in range(clo, chi):
                                    rhs = bass.AP(
                                        xq[:].tensor,
                                        xq[:].offset + off0 + ch * 512,
                                        [[pstep, 128], [2 * PW, 2], [1, 512]])
                                    nc.tensor.matmul(
                                        pq[:, (ch - clo) * 512:(ch - clo + 1) * 512],
                                        lhs, rhs, start=(nmm == 0),
                                        stop=(nmm == 13),
                                        perf_mode=mybir.MatmulPerfMode.DoubleRow)
                                nmm += 1
                    for (pq, clo, chi, nk) in halves:
                        # fused stream: pi-swapped lanes
                        fv = fus[:, clo * 448:chi * 448] \
                            .rearrange("p (k r w) -> p k r w", k=nk, r=8, w=56)
                        nc.scalar.activation(
                            fv[fulo:fulo + 64],
                            _cmp_half(pq[:], nk)[fulo:fulo + 64],
                            AF.Copy, scale=1.0 / 1024.0)
                        # c3 stream drained on ACT too (PE must not wait on DVE)
                        cv = c3t[:, clo * 448:chi * 448] \
                            .rearrange("p (k r w) -> p k r w", k=nk, r=8, w=56)
                        nc.scalar.activation(
                            cv[c3lo:c3lo + 64],
                            _cmp_half(pq[:], nk)[c3lo:c3lo + 64],
                            AF.Copy, scale=1.0 / 1024.0)

                nc.vector.tensor_tensor(y0[:], c3t[:], xp[:], OP.add)
                if b3_nonzero:
                    nc.vector.tensor_scalar(
                        y0[:], y0[:], b3_sb[:, blk:blk + 1], None, OP.add)
                if b12_nonzero:
                    nc.vector.tensor_scalar(
                        fus[:], fus[:], bf_sb[:, blk:blk + 1], None, OP.add)

                # ---- fus8 (padded, fp8, x128) for the score conv ----
                fus8 = fus8_pool.tile([128, PLANE], fp8)
                nc.gpsimd.memset(fus8[:, 0:3 * PW], 0.0)
                nc.gpsimd.memset(fus8[:, 59 * PW:PLANE], 0.0)
                f8col = fus8[:, 3 * PW:59 * PW].rearrange("p (h w) -> p h w", w=PW)
                nc.gpsimd.memset(f8col[:, :, 0:4], 0.0)
                nc.gpsimd.memset(f8col[:, :, 60:64], 0.0)
                nc.scalar.activation(_pad_view(fus8, 0, 0),
                                     fus[:].rearrange("p (k r w) -> p k r w",
                                                      k=7, r=8, w=56),
                                     AF.Copy, scale=128.0)

                # ---- score conv 3x3 (diag, pi order), one chunk at a time ----
                scr = scr_pool.tile([128, FLT], f32)
                f8step = fus8[:].ap[0][0]
                for ch in range(7):
                    hp = pepA_pool if ch % 2 == 0 else pepB_pool
                    sp = hp.tile([128, 512], f32, tag=f"pep{ch % 2}",
                                 name=f"sp{t}_{ch}")
                    for pi2, dx in enumerate(range(-1, 2)):
                        base = (blk * 9 + 2 * pi2) * 128
                        lhs = dgS_sb[:, base:base + 256] \
                            .rearrange("p (i m) -> p i m", i=2, m=128)
                        off0 = ORIG - PW + dx
                        rhs = bass.AP(fus8[:].tensor,
                                      fus8[:].offset + off0 + ch * 512,
                                      [[f8step, 128], [PW, 2], [1, 512]])
                        nc.tensor.matmul(sp[:], lhs, rhs,
                                         start=(pi2 == 0), stop=False,
                                         perf_mode=mybir.MatmulPerfMode.DoubleRow)
                    for si, dx in enumerate(range(-1, 2)):
                        base = (blk * 9 + 6 + si) * 128
                        lhs = dgS_sb[:, base:base + 128]
                        off0 = ORIG + PW + dx
                        rhs = fus8[:, off0 + ch * 512: off0 + ch * 512 + 512]
                        nc.tensor.matmul(sp[:], lhs, rhs,
                                         start=False, stop=(si == 2))
                    nc.scalar.activation(
                        scr[:, ch * 448:(ch + 1) * 448]
                        .rearrange("p (r w) -> p r w", r=8, w=56),
                        sp[:].rearrange("p (r w) -> p r w", r=8, w=64)[:, :, :56],
                        AF.Copy)

                # ---- top-30 threshold ----
                scrw = scrw_pool.tile([128, FLT], f32, tag="scrw", name=f"scrw{t}")
                rv = [sm_pool.tile([128, 8], f32, tag="rv", name=f"rv{t}_{r}")
                      for r in range(4)]
                nc.vector.max(rv[0][:], scr[:])
                nc.vector.match_replace(scrw[:], rv[0][:], scr[:], NEG_BIG)
                for r in (1, 2):
                    nc.vector.max(rv[r][:], scrw[:])
                    nc.vector.match_replace(scrw[:], rv[r][:], scrw[:], NEG_BIG)
                nc.vector.max(rv[3][:], scrw[:])

                # mask = (scr >= rank30) * fus   (pi order)
                msk = msk_pool.tile([128, FLT], bf16, tag="msk", name=f"msk{t}")
                nc.vector.tensor_scalar(msk[:], scr[:],
                                        rv[3][:, 5:6], None, OP.is_ge)
                nc.vector.tensor_tensor(msk[:], msk[:], fus[:], OP.mult)

                # un-permute the masked product: swap halves via DMA
                mskT = mskT_pool.tile([128, FLT], bf16, tag="mskT", name=f"mskT{t}")
                nc.sync.dma_start(out=mskT[0:64, :], in_=msk[64:128, :])
                nc.sync.dma_start(out=mskT[64:128, :], in_=msk[0:64, :])

                # ---- y = o1 + y0 ; gsum ----
                gs = gs_pool.tile([128, 1], f32)
                nc.vector.scalar_tensor_tensor(
                    y0[:], mskT[:], 1.0, y0[:],
                    OP.mult, OP.add, accum_out=gs[:])
                gsums[t] = gs

                if t >= 2 and blk == 0:
                    emit_se_a(t, (t - 2) // NBLK)
                    done_a.add((t - 2) // NBLK)
                if t >= 3 and blk == 1:
                    emit_se_b(t, (t - 3) // NBLK)
                    done_b.add((t - 3) // NBLK)
            tt = ntiles + 1
            for bd in range(b_loc):
                if bd not in done_a:
                    emit_se_a(tt, bd)
                    tt += 1
                if bd not in done_b:
                    emit_se_b(tt, bd)
                    tt += 1

    nc.compile()
    return nc


def mybir_np_fp8():
    import concourse.mybir as mybir
    return mybir.dt.np(mybir.dt.float8e4)


def _host_prep(inputs):
    x = np.ascontiguousarray(inputs["x"], dtype=np.float32)
    w1 = np.asarray(inputs["w1"], dtype=np.float32)
    b1 = np.asarray(inputs["b1"], dtype=np.float32)
    w2 = np.asarray(inputs["w2"], dtype=np.float32)
    b2 = np.asarray(inputs["b2"], dtype=np.float32)
    w3 = np.asarray(inputs["w3"], dtype=np.float32)
    b3 = np.asarray(inputs["b3"], dtype=np.float32)
    ws = np.asarray(inputs["ws"], dtype=np.float32)
    se_w1 = np.asarray(inputs["se_w1"], dtype=np.float32)
    se_w2 = np.asarray(inputs["se_w2"], dtype=np.float32)
    alpha = float(np.asarray(inputs["alpha"]))

    a = float(1.0 / (1.0 + np.exp(-alpha)))

    w12 = w2.copy()
    w12[:, :, 1:4, 1:4] += w1
    w12 = (a * w12)[:, 0]                     # [C,5,5]
    b12 = a * (b1 + b2)
    w3p = ((1.0 - a) * w3)[:, 0]              # [C,7,7]
    b3p = (1.0 - a) * b3
    wsf = ws[:, 0]                            # [C,3,3]

    f8m = mybir_np_fp8()

    # quad weights: per (blk, sb, col x, quad q): lhs [128 rows, 2 dr, 128 cols]
    # row p: group g=p//64, ch_local=p%64; dy = d + g + 2*i ; col j:
    #   c3 lanes [64*sb, 64*sb+64): ch j-64*sb ; fused lanes swapped half.
    dQ = np.zeros((NBLK, 2, 14, 128, 2, 128), dtype=np.float32)
    for blk in range(NBLK):
        for sb in range(2):
            c0 = blk * 128 + 64 * sb
            c3lo = 64 * sb
            fulo = 64 - 64 * sb
            for xi, xcol in enumerate(QCOLS):
                for qi, d in enumerate(QBASE):
                    slot = xi + 7 * qi
                    for g in range(2):
                        for i in range(2):
                            dy = d + g + 2 * i
                            for cl in range(64):
                                ch = c0 + cl
                                p = 64 * g + cl
                                if -3 <= dy <= 3:
                                    dQ[blk, sb, slot, p, i, c3lo + cl] = \
                                        w3p[ch, dy + 3, xcol + 3] * 1024.0
                                if -2 <= dy <= 2 and -2 <= xcol <= 2:
                                    dQ[blk, sb, slot, p, i, fulo + cl] = \
                                        w12[ch, dy + 2, xcol + 2] * 1024.0
    dgQ = np.ascontiguousarray(
        dQ.reshape(NBLK, 2 * 14, 128, 2 * 128).transpose(0, 2, 1, 3)
        .reshape(NBLK, 128, 2 * 14 * 2 * 128).astype(f8m))

    # score conv weights, channel order pi (swapped halves within each blk)
    pi_ws = wsf.reshape(NBLK, 2, 64, 3, 3)[:, ::-1].reshape(C, 3, 3)
    dS = np.zeros((NBLK, 128, 9, 128), dtype=np.float32)
    blkv, chv = np.divmod(np.arange(C), 128)
    for pi2, dx in enumerate(range(-1, 2)):
        for i in (0, 1):
            dS[blkv, chv, 2 * pi2 + i, chv] = pi_ws[:, i, dx + 1] * 1024.0
    for si, dx in enumerate(range(-1, 2)):
        dS[blkv, chv, 6 + si, chv] = pi_ws[:, 2, dx + 1] * 1024.0
    dgS = np.ascontiguousarray(dS.reshape(NBLK, 128, 9 * 128).astype(f8m))

    # fused-copy bias in pi order (b12 already carries the sigmoid(alpha) factor)
    bfpi = b12.reshape(NBLK, 2, 64)[:, ::-1].reshape(NBLK, 128, 1)

    s1 = (se_w1 / float(H * W)).T.reshape(NBLK, 128, 16)
    s2 = se_w2.T.reshape(16, NBLK, 128).transpose(1, 0, 2)

    common = {
        "dgQ": dgQ, "dgS": dgS,
        "bfpi": np.ascontiguousarray(bfpi, np.float32),
        "b3p": np.ascontiguousarray(b3p.reshape(NBLK, 128, 1), np.float32),
        "sew1": np.ascontiguousarray(s1, np.float32),
        "sew2": np.ascontiguousarray(s2, np.float32),
    }
    return x, common, bool(np.any(b3p != 0.0)), bool(np.any(b12 != 0.0))


def kernel(**inputs):
    from concourse.bass_utils import run_bass_kernel_spmd

    x, common, b3nz, b12nz = _host_prep(inputs)
    nc = build_nc(B_LOC, b3nz, b12nz)

    in_maps = []
    for i in range(N_CORES):
        m = {"x": np.ascontiguousarray(x[i * B_LOC:(i + 1) * B_LOC])}
        m.update(common)
        in_maps.append(m)

    res = run_bass_kernel_spmd(nc, in_maps, core_ids=list(range(N_CORES)))
    LAST.clear()
    LAST["exec_time_ns"] = res.exec_time_ns
    LAST["mean_exec_time_ns"] = res.mean_exec_time_ns
    out = np.concatenate([res.results[i]["out"] for i in range(N_CORES)], axis=0)
    return out


# revision 13
# speedup vs baseline: 1.0745x; 1.0745x over previous
"""Trainium2 Bass kernel for MineralFusion, v2 (quad-packed convs).

Key idea vs baseline: each fp8 DoubleRow matmul covers a 4-tap column-quad
(dy in {d,d+1,d+2,d+3}, fixed dx) for 64 channels of BOTH the fused 5x5 and
the 7x7 conv at once, via a channel-replica moving tile whose upper 64
partitions hold the same 64 channels pre-shifted by one image row (+PW).
DR step 128 (2 rows) covers dy = d, d+2 on the base copy and d+1, d+3 on the
shifted copy.  14 quad-mms per 64-ch sub-block per 512-col chunk replace the
~18 diag mms + 7 DVE taps of the baseline.  Output lanes: one sub-block's
c3 lands channel-aligned, the fused stream lands half-swapped (pi = XOR 64);
the pi order is carried through score conv / top-k / mask and undone by one
SBUF->SBUF DMA of the bf16 mask per tile.

Top-k: 4 rounds of max8 + 3 match_replace (into a scratch copy) give the
30th-largest value per row; mask = (scr >= t30) via one tensor_scalar pass.
"""
import numpy as np
import ml_dtypes

B, C, H, W = 32, 256, 56, 56
K = 30
N_CORES = 8
B_LOC = B // N_CORES
NBLK = C // 128

PW = 64
NROW = 62                     # 3 + 56 + 3 rows
PLANE = NROW * PW             # 3968
XF8_X = PLANE + PW            # xf8 carries slack for the +PW replica copy
XQ_X = PLANE + 8              # quad tiles: AP tail slack
XP_X = 3136
ORIG = 3 * PW + 4             # interior origin (row 3, col 4)
CMP = 3584                    # 7 chunks x 512 (448 data + 64 junk)
FLT = 3136                    # flat compact (no junk cols)
NEG_BIG = -(2.0 ** 100)

QCOLS = list(range(-3, 4))    # dx of quad columns
QBASE = (-3, 1)               # quad q covers dy in {d, d+1, d+2, d+3}

LAST = {}


def _pad_view(ap_flat, dy, dx):
    off = ORIG + dy * PW + dx
    v = ap_flat[:, off:off + 7 * 8 * PW]
    return v.rearrange("p (k r w) -> p k r w", k=7, r=8, w=PW)[:, :, :, :56]


def _cmp_half(ap_flat, nk):
    v = ap_flat.rearrange("p (k r w) -> p k r w", k=nk, r=8, w=64)
    return v[:, :, :, :56]


def _cmp_view(ap_flat):
    v = ap_flat.rearrange("p (k r w) -> p k r w", k=7, r=8, w=64)
    return v[:, :, :, :56]


def build_nc(b_loc=B_LOC, b3_nonzero=False, b12_nonzero=False):
    import concourse.bass as bass
    import concourse.mybir as mybir
    from concourse import bacc, tile

    f32 = mybir.dt.float32
    bf16 = mybir.dt.bfloat16
    fp8 = mybir.dt.float8e4
    AF = mybir.ActivationFunctionType
    OP = mybir.AluOpType

    ntiles = b_loc * NBLK

    nc = bacc.Bacc("TRN2", target_bir_lowering=False, debug=False)

    x_d = nc.declare_dram_parameter("x", [b_loc, C, H, W], f32, isOutput=False)
    # quad weights: [blk][sb][14 slots][2 dr][128 rows] -> as [128, NBLK*2*14*2*128]
    dgQ_d = nc.declare_dram_parameter("dgQ", [NBLK, 128, 2 * 14 * 2 * 128], fp8,
                                      isOutput=False)
    dgS_d = nc.declare_dram_parameter("dgS", [NBLK, 128, 9 * 128], fp8,
                                      isOutput=False)
    bf_d = nc.declare_dram_parameter("bfpi", [NBLK, 128, 1], f32, isOutput=False)
    b3_d = nc.declare_dram_parameter("b3p", [NBLK, 128, 1], f32, isOutput=False)
    s1_d = nc.declare_dram_parameter("sew1", [NBLK, 128, 16], f32, isOutput=False)
    s2_d = nc.declare_dram_parameter("sew2", [NBLK, 16, 128], f32, isOutput=False)
    out_d = nc.declare_dram_parameter("out", [b_loc, C, H, W], f32, isOutput=True)

    from contextlib import ExitStack
    with tile.TileContext(nc) as tc:
        with ExitStack() as stack:
            ep = stack.enter_context
            wpool = ep(tc.tile_pool(name="wpool", bufs=1))
            xp_pool = ep(tc.tile_pool(name="xp", bufs=2))
            xf8_pool = ep(tc.tile_pool(name="xf8", bufs=2))
            xq_pool = ep(tc.tile_pool(name="xq", bufs=1))
            c3t_pool = ep(tc.tile_pool(name="c3t", bufs=3))
            fus_pool = ep(tc.tile_pool(name="fus", bufs=3))
            fus8_pool = ep(tc.tile_pool(name="fus8", bufs=2))
            scr_pool = ep(tc.tile_pool(name="scr", bufs=3))
            scrw_pool = ep(tc.tile_pool(name="scrw", bufs=1))
            msk_pool = ep(tc.tile_pool(name="msk", bufs=1))
            mskT_pool = ep(tc.tile_pool(name="mskT", bufs=1))
            y0_pool = ep(tc.tile_pool(name="y0", bufs=6))
            sm_pool = ep(tc.tile_pool(name="small", bufs=8))
            gs_pool = ep(tc.tile_pool(name="gs", bufs=9))
            gate_pool = ep(tc.tile_pool(name="gate", bufs=6))
            hsb_pool = ep(tc.tile_pool(name="hsb", bufs=4))
            out_pool = ep(tc.tile_pool(name="outp", bufs=1))
            pepA_pool = ep(tc.tile_pool(name="pepA", bufs=1, space="PSUM"))
            pepB_pool = ep(tc.tile_pool(name="pepB", bufs=1, space="PSUM"))
            ps_pool = ep(tc.tile_pool(name="ps", bufs=1, space="PSUM"))
            dgQ_sb = wpool.tile([128, NBLK * 2 * 14 * 2 * 128], fp8)
            dgS_sb = wpool.tile([128, NBLK * 9 * 128], fp8)
            bf_sb = wpool.tile([128, NBLK], f32)
            b3_sb = wpool.tile([128, NBLK], f32)
            s1_sb = wpool.tile([128, NBLK * 16], f32)
            s2_sb = wpool.tile([16, NBLK * 128], f32)
            QW = 2 * 14 * 2 * 128
            for blk in range(NBLK):
                nc.sync.dma_start(out=dgQ_sb[:, blk * QW:(blk + 1) * QW],
                                  in_=dgQ_d[blk])
                nc.sync.dma_start(out=dgS_sb[:, blk * 9 * 128:(blk + 1) * 9 * 128],
                                  in_=dgS_d[blk])
                nc.sync.dma_start(out=bf_sb[:, blk:blk + 1], in_=bf_d[blk])
                nc.sync.dma_start(out=b3_sb[:, blk:blk + 1], in_=b3_d[blk])
                nc.sync.dma_start(out=s1_sb[:, blk * 16:(blk + 1) * 16], in_=s1_d[blk])
                nc.sync.dma_start(out=s2_sb[:, blk * 128:(blk + 1) * 128],
                                  in_=s2_d[blk])

            gsums = {}
            y0s = {}
            hsbs = {}
            done_a = set()
            done_b = set()

            def emit_se_a(t, bd):
                hp = ps_pool.tile([16, 1], f32, tag="sep", name=f"hp{t}")
                for b2 in range(NBLK):
                    nc.tensor.matmul(
                        hp[:], s1_sb[:, b2 * 16:(b2 + 1) * 16],
                        gsums[bd * NBLK + b2][:],
                        start=(b2 == 0), stop=(b2 == NBLK - 1))
                hsb = hsb_pool.tile([16, 1], f32, tag="hsb", name=f"hsb{t}")
                nc.scalar.activation(hsb[:], hp[:], AF.Relu)
                hsbs[bd] = hsb

            def emit_se_b(t, bd):
                hsb = hsbs[bd]
                for b2 in range(NBLK):
                    glp = ps_pool.tile([128, 1], f32, tag="sep", name=f"glp{t}_{b2}")
                    nc.tensor.matmul(
                        glp[:], s2_sb[:, b2 * 128:(b2 + 1) * 128], hsb[:],
                        start=True, stop=True)
                    gt = gate_pool.tile([128, 1], f32, tag="gate", name=f"gt{t}_{b2}")
                    nc.scalar.activation(gt[:], glp[:], AF.Sigmoid)
                    nc.vector.tensor_scalar_add(gt[:], gt[:], 1.0)
                    t2 = bd * NBLK + b2
                    dst = out_d[bd, b2 * 128:(b2 + 1) * 128] \
                        .rearrange("c h w -> c (h w)")
                    for hf in range(2):
                        outf = out_pool.tile([128, FLT // 2], f32, tag="outp",
                                             name=f"outf{t}_{b2}_{hf}")
                        sl = slice(hf * (FLT // 2), (hf + 1) * (FLT // 2))
                        nc.scalar.activation(outf[:], y0s[t2][:, sl],
                                             AF.Copy, bias=0.0, scale=gt[:])
                        nc.sync.dma_start(out=dst[:, sl], in_=outf[:])

            for t in range(ntiles):
                b, blk = divmod(t, NBLK)
                c0 = blk * 128

                xp = xp_pool.tile([128, XP_X], f32)
                x_src = x_d[b, c0:c0 + 128].rearrange("c h w -> c (h w)")
                nc.sync.dma_start(out=xp[:], in_=x_src)

                xf8 = xf8_pool.tile([128, XF8_X], fp8)
                nc.gpsimd.memset(xf8[:, PLANE:XF8_X], 0.0)
                nc.gpsimd.memset(xf8[:, 0:3 * PW], 0.0)
                nc.gpsimd.memset(xf8[:, 59 * PW:PLANE], 0.0)
                xcol = xf8[:, 3 * PW:59 * PW].rearrange("p (h w) -> p h w", w=PW)
                nc.gpsimd.memset(xcol[:, :, 0:4], 0.0)
                nc.gpsimd.memset(xcol[:, :, 60:64], 0.0)
                nc.scalar.activation(
                    _pad_view(xf8, 0, 0),
                    xp[:].rearrange("p (k r w) -> p k r w", k=7, r=8, w=56),
                    AF.Copy)

                # channel-replica quad tiles: xq[sb][0:64) = xf8 sub-block,
                # [64:128) = same shifted by one image row (+PW)
                xqs = []
                for sb in range(2):
                    xq = xq_pool.tile([128, XQ_X], fp8, tag=f"xq{sb}", name=f"xq{t}_{sb}")
                    nc.gpsimd.memset(xq[:, PLANE:XQ_X], 0.0)
                    nc.sync.dma_start(out=xq[0:64, 0:PLANE],
                                      in_=xf8[64 * sb:64 * sb + 64, 0:PLANE])
                    nc.sync.dma_start(out=xq[64:128, 0:PLANE],
                                      in_=xf8[64 * sb:64 * sb + 64, PW:PLANE + PW])
                    xqs.append(xq)

                # ---- quad conv phases: sb0 then sb1 ----
                fus = fus_pool.tile([128, FLT], bf16)
                c3t = c3t_pool.tile([128, FLT], bf16, tag="c3t", name=f"c3t{t}")
                y0 = y0_pool.tile([128, FLT], bf16)
                y0s[t] = y0
                pstep = xqs[0][:].ap[0][0]
                for sb in range(2):
                    xq = xqs[sb]
                    c3lo = 64 * sb            # c3 lanes: channel-aligned
                    fulo = 64 - 64 * sb       # fused lanes: swapped half
                    halves = []
                    for hi, (hpool, clo, chi, nk) in enumerate(
                            ((pepA_pool, 0, 4, 4), (pepB_pool, 4, 7, 3))):
                        pq = hpool.tile([128, nk * 512], f32, tag=f"pep{hi}",
                                        name=f"pq{t}_{sb}_{hi}")
                        halves.append((pq, clo, chi, nk))
                        nmm = 0
                        for x in QCOLS:
                            for qi, d in enumerate(QBASE):
                                base = (blk * 2 + sb) * 14 + x + 3 + 7 * qi
                                lhs = dgQ_sb[:, base * 256:(base + 1) * 256] \
                                    .rearrange("p (i m) -> p i m", i=2, m=128)
                                off0 = ORIG + d * PW + x
                                for ch in range(clo, chi):
                                    rhs = bass.AP(
                                        xq[:].tensor,
                                        xq[:].offset + off0 + ch * 512,
                                        [[pstep, 128], [2 * PW, 2], [1, 512]])
                                    nc.tensor.matmul(
                                        pq[:, (ch - clo) * 512:(ch - clo + 1) * 512],
                                        lhs, rhs, start=(nmm == 0),
                                        stop=(nmm == 13),
                                        perf_mode=mybir.MatmulPerfMode.DoubleRow)
                                nmm += 1
                    for (pq, clo, chi, nk) in halves:
                        # fused stream: pi-swapped lanes
                        fv = fus[:, clo * 448:chi * 448] \
                            .rearrange("p (k r w) -> p k r w", k=nk, r=8, w=56)
                        nc.scalar.activation(
                            fv[fulo:fulo + 64],
                            _cmp_half(pq[:], nk)[fulo:fulo + 64],
                            AF.Copy, scale=1.0 / 1024.0)
                        # c3 stream drained on ACT too (PE must not wait on DVE)
                        cv = c3t[:, clo * 448:chi * 448] \
                            .rearrange("p (k r w) -> p k r w", k=nk, r=8, w=56)
                        nc.scalar.activation(
                            cv[c3lo:c3lo + 64],
                            _cmp_half(pq[:], nk)[c3lo:c3lo + 64],
                            AF.Copy, scale=1.0 / 1024.0)

                nc.vector.tensor_tensor(y0[:], c3t[:], xp[:], OP.add)
                if b3_nonzero:
                    nc.vector.tensor_scalar(
                        y0[:], y0[:], b3_sb[:, blk:blk + 1], None, OP.add)
                if b12_nonzero:
                    nc.vector.tensor_scalar(
                        fus[:], fus[:], bf_sb[:, blk:blk + 1], None, OP.add)

                # ---- fus8 (padded, fp8, x128) for the score conv ----
                fus8 = fus8_pool.tile([128, PLANE], fp8)
                nc.gpsimd.memset(fus8[:, 0:3 * PW], 0.0)
                nc.gpsimd.memset(fus8[:, 59 * PW:PLANE], 0.0)
                f8col = fus8[:, 3 * PW:59 * PW].rearrange("p (h w) -> p h w", w=PW)
                nc.gpsimd.memset(f8col[:, :, 0:4], 0.0)
                nc.gpsimd.memset(f8col[:, :, 60:64], 0.0)
                nc.scalar.activation(_pad_view(fus8, 0, 0),
                                     fus[:].rearrange("p (k r w) -> p k r w",
                                                      k=7, r=8, w=56),
                                     AF.Copy, scale=128.0)

                # ---- score conv 3x3 (diag, pi order), one chunk at a time ----
                scr = scr_pool.tile([128, FLT], f32)
                f8step = fus8[:].ap[0][0]
                for ch in range(7):
                    sp = ps_pool.tile([128, 512], f32, tag="sep",
                                      name=f"sp{t}_{ch}")
                    for pi2, dx in enumerate(range(-1, 2)):
                        base = (blk * 9 + 2 * pi2) * 128
                        lhs = dgS_sb[:, base:base + 256] \
                            .rearrange("p (i m) -> p i m", i=2, m=128)
                        off0 = ORIG - PW + dx
                        rhs = bass.AP(fus8[:].tensor,
                                      fus8[:].offset + off0 + ch * 512,
                                      [[f8step, 128], [PW, 2], [1, 512]])
                        nc.tensor.matmul(sp[:], lhs, rhs,
                                         start=(pi2 == 0), stop=False,
                                         perf_mode=mybir.MatmulPerfMode.DoubleRow)
                    for si, dx in enumerate(range(-1, 2)):
                        base = (blk * 9 + 6 + si) * 128
                        lhs = dgS_sb[:, base:base + 128]
                        off0 = ORIG + PW + dx
                        rhs = fus8[:, off0 + ch * 512: off0 + ch * 512 + 512]
                        nc.tensor.matmul(sp[:], lhs, rhs,
                                         start=False, stop=(si == 2))
                    nc.scalar.activation(
                        scr[:, ch * 448:(ch + 1) * 448]
                        .rearrange("p (r w) -> p r w", r=8, w=56),
                        sp[:].rearrange("p (r w) -> p r w", r=8, w=64)[:, :, :56],
                        AF.Copy)

                # ---- top-30 threshold ----
                scrw = scrw_pool.tile([128, FLT], f32, tag="scrw", name=f"scrw{t}")
                rv = [sm_pool.tile([128, 8], f32, tag="rv", name=f"rv{t}_{r}")
                      for r in range(4)]
                nc.vector.max(rv[0][:], scr[:])
                nc.vector.match_replace(scrw[:], rv[0][:], scr[:], NEG_BIG)
                for r in (1, 2):
                    nc.vector.max(rv[r][:], scrw[:])
                    nc.vector.match_replace(scrw[:], rv[r][:], scrw[:], NEG_BIG)
                nc.vector.max(rv[3][:], scrw[:])

                # mask = (scr >= rank30) * fus   (pi order)
                msk = msk_pool.tile([128, FLT], bf16, tag="msk", name=f"msk{t}")
                nc.vector.tensor_scalar(msk[:], scr[:],
                                        rv[3][:, 5:6], None, OP.is_ge)
                nc.vector.tensor_tensor(msk[:], msk[:], fus[:], OP.mult)

                # un-permute the masked product: swap halves via DMA
                mskT = mskT_pool.tile([128, FLT], bf16, tag="mskT", name=f"mskT{t}")
                nc.sync.dma_start(out=mskT[0:64, :], in_=msk[64:128, :])
                nc.sync.dma_start(out=mskT[64:128, :], in_=msk[0:64, :])

                # ---- y = o1 + y0 ; gsum ----
                gs = gs_pool.tile([128, 1], f32)
                nc.vector.scalar_tensor_tensor(
                    y0[:], mskT[:], 1.0, y0[:],
                    OP.mult, OP.add, accum_out=gs[:])
                gsums[t] = gs

                if t >= 2 and blk == 0:
                    emit_se_a(t, (t - 2) // NBLK)
                    done_a.add((t - 2) // NBLK)
                if t >= 3 and blk == 1:
                    emit_se_b(t, (t - 3) // NBLK)
                    done_b.add((t - 3) // NBLK)
            tt = ntiles + 1
            for bd in range(b_loc):
                if bd not in done_a:
                    emit_se_a(tt, bd)
                    tt += 1
                if bd not in done_b:
                    emit_se_b(tt, bd)
                    tt += 1

    nc.compile()
    return nc


def mybir_np_fp8():
    import concourse.mybir as mybir
    return mybir.dt.np(mybir.dt.float8e4)


def _host_prep(inputs):
    x = np.ascontiguousarray(inputs["x"], dtype=np.float32)
    w1 = np.asarray(inputs["w1"], dtype=np.float32)
    b1 = np.asarray(inputs["b1"], dtype=np.float32)
    w2 = np.asarray(inputs["w2"], dtype=np.float32)
    b2 = np.asarray(inputs["b2"], dtype=np.float32)
    w3 = np.asarray(inputs["w3"], dtype=np.float32)
    b3 = np.asarray(inputs["b3"], dtype=np.float32)
    ws = np.asarray(inputs["ws"], dtype=np.float32)
    se_w1 = np.asarray(inputs["se_w1"], dtype=np.float32)
    se_w2 = np.asarray(inputs["se_w2"], dtype=np.float32)
    alpha = float(np.asarray(inputs["alpha"]))

    a = float(1.0 / (1.0 + np.exp(-alpha)))

    w12 = w2.copy()
    w12[:, :, 1:4, 1:4] += w1
    w12 = (a * w12)[:, 0]                     # [C,5,5]
    b12 = a * (b1 + b2)
    w3p = ((1.0 - a) * w3)[:, 0]              # [C,7,7]
    b3p = (1.0 - a) * b3
    wsf = ws[:, 0]                            # [C,3,3]

    f8m = mybir_np_fp8()

    # quad weights: per (blk, sb, col x, quad q): lhs [128 rows, 2 dr, 128 cols]
    # row p: group g=p//64, ch_local=p%64; dy = d + g + 2*i ; col j:
    #   c3 lanes [64*sb, 64*sb+64): ch j-64*sb ; fused lanes swapped half.
    dQ = np.zeros((NBLK, 2, 14, 128, 2, 128), dtype=np.float32)
    for blk in range(NBLK):
        for sb in range(2):
            c0 = blk * 128 + 64 * sb
            c3lo = 64 * sb
            fulo = 64 - 64 * sb
            for xi, xcol in enumerate(QCOLS):
                for qi, d in enumerate(QBASE):
                    slot = xi + 7 * qi
                    for g in range(2):
                        for i in range(2):
                            dy = d + g + 2 * i
                            for cl in range(64):
                                ch = c0 + cl
                                p = 64 * g + cl
                                if -3 <= dy <= 3:
                                    dQ[blk, sb, slot, p, i, c3lo + cl] = \
                                        w3p[ch, dy + 3, xcol + 3] * 1024.0
                                if -2 <= dy <= 2 and -2 <= xcol <= 2:
                                    dQ[blk, sb, slot, p, i, fulo + cl] = \
                                        w12[ch, dy + 2, xcol + 2] * 1024.0
    dgQ = np.ascontiguousarray(
        dQ.reshape(NBLK, 2 * 14, 128, 2 * 128).transpose(0, 2, 1, 3)
        .reshape(NBLK, 128, 2 * 14 * 2 * 128).astype(f8m))

    # score conv weights, channel order pi (swapped halves within each blk)
    pi_ws = wsf.reshape(NBLK, 2, 64, 3, 3)[:, ::-1].reshape(C, 3, 3)
    dS = np.zeros((NBLK, 128, 9, 128), dtype=np.float32)
    blkv, chv = np.divmod(np.arange(C), 128)
    for pi2, dx in enumerate(range(-1, 2)):
        for i in (0, 1):
            dS[blkv, chv, 2 * pi2 + i, chv] = pi_ws[:, i, dx + 1] * 1024.0
    for si, dx in enumerate(range(-1, 2)):
        dS[blkv, chv, 6 + si, chv] = pi_ws[:, 2, dx + 1] * 1024.0
    dgS = np.ascontiguousarray(dS.reshape(NBLK, 128, 9 * 128).astype(f8m))

    # fused-copy bias in pi order (b12 already carries the sigmoid(alpha) factor)
    bfpi = b12.reshape(NBLK, 2, 64)[:, ::-1].reshape(NBLK, 128, 1)

    s1 = (se_w1 / float(H * W)).T.reshape(NBLK, 128, 16)
    s2 = se_w2.T.reshape(16, NBLK, 128).transpose(1, 0, 2)

    common = {
        "dgQ": dgQ, "dgS": dgS,
        "bfpi": np.ascontiguousarray(bfpi, np.float32),
        "b3p": np.ascontiguousarray(b3p.reshape(NBLK, 128, 1), np.float32),
        "sew1": np.ascontiguousarray(s1, np.float32),
        "sew2": np.ascontiguousarray(s2, np.float32),
    }
    return x, common, bool(np.any(b3p != 0.0)), bool(np.any(b12 != 0.0))


def kernel(**inputs):
    from concourse.bass_utils import run_bass_kernel_spmd

    x, common, b3nz, b12nz = _host_prep(inputs)
    nc = build_nc(B_LOC, b3nz, b12nz)

    in_maps = []
    for i in range(N_CORES):
        m = {"x": np.ascontiguousarray(x[i * B_LOC:(i + 1) * B_LOC])}
        m.update(common)
        in_maps.append(m)

    res = run_bass_kernel_spmd(nc, in_maps, core_ids=list(range(N_CORES)))
    LAST.clear()
    LAST["exec_time_ns"] = res.exec_time_ns
    LAST["mean_exec_time_ns"] = res.mean_exec_time_ns
    out = np.concatenate([res.results[i]["out"] for i in range(N_CORES)], axis=0)
    return out
